# revision 16
# baseline (speedup 1.0000x reference)
"""Trainium2 Bass kernel for nn_EventADModel (2-layer event GRU + coord GRU + fusion MLP).

Strategy
--------
Pure data parallel across 8 NeuronCores: shard the B*T = 245760 (b,t) "tracks"
into 8 shards of 30720. All weights are replicated.

On-chip layout: hidden/gate dim on SBUF partitions, tracks on the free dim.
Host (numpy, free) pre-transposes inputs to [feature, N] fp16, pre-collapses
the fusion MLP (W1a@We, W1b@Wc), and applies b2 + valid mask to the device
output.  Step-1 GRU algebra (h==0) skips the hidden-state matmuls; step-1
outputs are kept NEGATED (-h = (z-1)*n computed in one fused op) with the
sign folded into the consumer weight copies on the host.

Matmuls run in fp16 (fp32 PSUM accumulation); sigmoid/tanh on the scalar
engine; gate combining split between the vector engine (PSUM-sourced ops)
and gpsimd (SBUF-only ops).
"""

import os
import sys

for _p in ("/opt/trn_rl_repo",):
    if os.path.isdir(_p) and _p not in sys.path:
        sys.path.insert(0, _p)

import numpy as np

F16 = np.float16

# Problem constants (hardcoded per contract).
B, F, T, X = 8192, 2, 30, 64
HE, HC = 256, 32
N_CORES = 8
N_TOT = B * T                 # 245760
NC_TRACKS = N_TOT // N_CORES  # 30720
NT = 256                      # tracks per main tile
G = 3 * HE                    # 768 gate rows

_CACHE = {}
LAST_RESULTS = None


def _pack_k(wT, m):
    """[k_tot, m] -> [128, (k_tot//128)*m] with K-chunks side by side."""
    kc = wT.shape[0] // 128
    return np.ascontiguousarray(
        wT.reshape(kc, 128, m).transpose(1, 0, 2).reshape(128, kc * m)
    )


def _build_program(zero_bias):
    import concourse.bacc as bacc
    import concourse.mybir as mybir
    from concourse import tile

    dt = mybir.dt
    AF = mybir.ActivationFunctionType
    OP = mybir.AluOpType

    nc = bacc.Bacc("TRN2", target_bir_lowering=False, debug=False,
                   num_devices=N_CORES)

    # ---- DRAM tensors -------------------------------------------------
    xt_d = nc.dram_tensor("xt", [128, NC_TRACKS], dt.float16, kind="ExternalInput")
    ct_d = nc.dram_tensor("ct", [4, NC_TRACKS], dt.float16, kind="ExternalInput")
    out_d = nc.dram_tensor("out", [NC_TRACKS, 2], dt.float32, kind="ExternalOutput")

    # Wih0.T duplicated on both partition halves so frame-0 matmuls use
    # rows 0:64 and frame-1 matmuls rows 64:128 (lhsT/rhs base must match).
    w0x_d = nc.dram_tensor("w0x", [128, G], dt.float16, kind="ExternalInput")
    # *_n variants are negated (consumers of negated h states).
    w0hn_d = nc.dram_tensor("w0hn", [128, 2 * G], dt.float16, kind="ExternalInput")
    w1x_d = nc.dram_tensor("w1x", [128, 2 * G], dt.float16, kind="ExternalInput")
    w1xn_d = nc.dram_tensor("w1xn", [128, 2 * G], dt.float16, kind="ExternalInput")
    w1hn_d = nc.dram_tensor("w1hn", [128, 2 * G], dt.float16, kind="ExternalInput")
    wc_d = nc.dram_tensor("wc", [4, 96], dt.float16, kind="ExternalInput")
    wae_d = nc.dram_tensor("wae", [128, 256], dt.float16, kind="ExternalInput")
    wacn_d = nc.dram_tensor("wacn", [32, 128], dt.float16, kind="ExternalInput")
    w2t_d = nc.dram_tensor("w2t", [128, 2], dt.float16, kind="ExternalInput")
    # biases packed [128, 18] f32: cols 0:4 rz0, 4:8 rz1, 8:10 bn0,
    # 10:12 bhn0, 12:14 bn1, 14:16 bhn1, 16 b_hid.
    bias_d = nc.dram_tensor("biases", [128, 18], dt.float32, kind="ExternalInput")
    biasc_d = nc.dram_tensor("biasc", [32, 8], dt.float32, kind="ExternalInput")

    TILES = NC_TRACKS // NT

    with tile.TileContext(nc) as tc:
        with (
            tc.tile_pool(name="wpool", bufs=1) as wp,
            tc.tile_pool(name="xin", bufs=4) as xin,
            tc.tile_pool(name="gate", bufs=3) as gp,
            tc.tile_pool(name="state", bufs=3) as sp,
            tc.tile_pool(name="outp", bufs=4) as op_,
            tc.tile_pool(name="psA", bufs=4, space="PSUM") as psA,
            tc.tile_pool(name="psB", bufs=4, space="PSUM") as psB,
        ):
            # ---- resident weights ------------------------------------
            w0x = wp.tile([128, G], dt.float16, name="w0x_s")
            w0hn = wp.tile([128, 2 * G], dt.float16, name="w0hn_s")
            w1x = wp.tile([128, 2 * G], dt.float16, name="w1x_s")
            w1xn = wp.tile([128, 2 * G], dt.float16, name="w1xn_s")
            w1hn = wp.tile([128, 2 * G], dt.float16, name="w1hn_s")
            wc = wp.tile([4, 96], dt.float16, name="wc_s")
            wae = wp.tile([128, 256], dt.float16, name="wae_s")
            wacn = wp.tile([32, 128], dt.float16, name="wacn_s")
            w2t = wp.tile([128, 2], dt.float16, name="w2t_s")
            bias = wp.tile([128, 18], dt.float32, name="bias_s")
            biasc = wp.tile([32, 8], dt.float32, name="biasc_s")
            for sb_t, dr in ((w0x, w0x_d), (w0hn, w0hn_d), (w1x, w1x_d),
                             (w1xn, w1xn_d), (w1hn, w1hn_d), (wc, wc_d),
                             (wae, wae_d), (wacn, wacn_d), (w2t, w2t_d),
                             (bias, bias_d), (biasc, biasc_d)):
                nc.sync.dma_start(sb_t[:], dr[:])

            def ps1(name):
                return psA.tile([128, 2 * NT], dt.float32, name=name, tag="psA")

            def ps2(name):
                return psB.tile([128, 2 * NT], dt.float32, name=name, tag="psB")

            def sig2(g_ps, bias_off, name):
                """sigmoid over a [128, 2*NT] psum pair -> fp16 sbuf."""
                outt = gp.tile([128, 2 * NT], dt.float16, name=name, tag=name)
                if zero_bias:
                    nc.scalar.activation(outt[:], g_ps[:], AF.Sigmoid)
                else:
                    for c in range(2):
                        sl = slice(c * NT, (c + 1) * NT)
                        nc.scalar.activation(
                            outt[:, sl], g_ps[:, sl], AF.Sigmoid,
                            bias=bias[:, bias_off + c:bias_off + c + 1])
                return outt

            def first_step(wxap, rhs_x, kc_x, boff_rz, boff_n, boff_hn, hname):
                """GRU step with h==0; returns NEGATED h = (z-1)*n."""
                gz = ps1("gz_f")
                for jj, j in enumerate((2, 3)):  # z chunks
                    for kc in range(kc_x):
                        nc.tensor.matmul(
                            gz[:, jj * NT:(jj + 1) * NT], wxap(kc, j),
                            rhs_x(kc), start=(kc == 0), stop=(kc == kc_x - 1))
                gn = ps2("gn_f")
                for jj, j in enumerate((4, 5)):  # n chunks
                    for kc in range(kc_x):
                        nc.tensor.matmul(
                            gn[:, jj * NT:(jj + 1) * NT], wxap(kc, j),
                            rhs_x(kc), start=(kc == 0), stop=(kc == kc_x - 1))
                z_s = sig2(gz, boff_rz + 2, "zs_" + hname)
                n_s = gp.tile([128, 2 * NT], dt.float16, name="ns_" + hname,
                              tag="ns_" + hname)
                if zero_bias:
                    nc.scalar.activation(n_s[:], gn[:], AF.Tanh)
                else:
                    gr = ps1("gr_f")
                    for jj, j in enumerate((0, 1)):  # r chunks
                        for kc in range(kc_x):
                            nc.tensor.matmul(
                                gr[:, jj * NT:(jj + 1) * NT], wxap(kc, j),
                                rhs_x(kc), start=(kc == 0),
                                stop=(kc == kc_x - 1))
                    r_s = sig2(gr, boff_rz, "rs_" + hname)
                    u = gp.tile([128, 2 * NT], dt.float16, name="u_" + hname,
                                tag="u_f")
                    for c in range(2):
                        sl = slice(c * NT, (c + 1) * NT)
                        t_c = gp.tile([128, NT], dt.float16, name="t_f", tag="t_f")
                        nc.vector.tensor_scalar_mul(
                            t_c[:], r_s[:, sl],
                            bias[:, boff_hn + c:boff_hn + c + 1])
                        nc.vector.scalar_tensor_tensor(
                            u[:, sl], gn[:, sl],
                            bias[:, boff_n + c:boff_n + c + 1],
                            t_c[:], OP.add, OP.add)
                    nc.scalar.activation(n_s[:], u[:], AF.Tanh)
                nh = sp.tile([128, 2 * NT], dt.float16, name=hname, tag=hname)
                # nh = (z-1)*n = -h
                nc.vector.scalar_tensor_tensor(nh[:], z_s[:], 1.0, n_s[:],
                                               OP.subtract, OP.mult)
                return nh

            def full_step(wxap, kc_x, rhs_x, wh, nh_prev, boff_rz, boff_n,
                          boff_hn, hname):
                """General GRU step; x-part via wxap, h-part via wh @ (-nh_prev)
                with wh already negated.  Returns POSITIVE h'."""
                gr = ps1("gr_g")
                gz = ps1("gz_g")
                for g_ps, j0 in ((gr, 0), (gz, 2)):
                    for jj in range(2):
                        j = j0 + jj
                        sl = slice(jj * NT, (jj + 1) * NT)
                        for kc in range(kc_x):
                            nc.tensor.matmul(
                                g_ps[:, sl], wxap(kc, j), rhs_x(kc),
                                start=(kc == 0), stop=False)
                        for kc in range(2):
                            nc.tensor.matmul(
                                g_ps[:, sl],
                                wh[:, kc * G + j * 128:kc * G + (j + 1) * 128],
                                nh_prev[:, kc * NT:(kc + 1) * NT],
                                start=False, stop=(kc == 1))
                gi = ps2("gi_g")
                gh = ps2("gh_g")
                for jj, j in enumerate((4, 5)):
                    sl = slice(jj * NT, (jj + 1) * NT)
                    for kc in range(kc_x):
                        nc.tensor.matmul(gi[:, sl], wxap(kc, j), rhs_x(kc),
                                         start=(kc == 0), stop=(kc == kc_x - 1))
                    for kc in range(2):
                        nc.tensor.matmul(
                            gh[:, sl],
                            wh[:, kc * G + j * 128:kc * G + (j + 1) * 128],
                            nh_prev[:, kc * NT:(kc + 1) * NT],
                            start=(kc == 0), stop=(kc == 1))
                r_s = sig2(gr, boff_rz, "rs_" + hname)
                z_s = sig2(gz, boff_rz + 2, "zs_" + hname)
                t = gp.tile([128, 2 * NT], dt.float16, name="t_" + hname, tag="t_g")
                u = gp.tile([128, 2 * NT], dt.float16, name="u_" + hname, tag="u_g")
                if zero_bias:
                    nc.vector.tensor_mul(t[:], r_s[:], gh[:])
                    nc.vector.tensor_add(u[:], t[:], gi[:])
                else:
                    for c in range(2):
                        sl = slice(c * NT, (c + 1) * NT)
                        nc.vector.scalar_tensor_tensor(
                            t[:, sl], gh[:, sl],
                            bias[:, boff_hn + c:boff_hn + c + 1],
                            r_s[:, sl], OP.add, OP.mult)
                        nc.vector.scalar_tensor_tensor(
                            u[:, sl], gi[:, sl],
                            bias[:, boff_n + c:boff_n + c + 1],
                            t[:, sl], OP.add, OP.add)
                n_s = gp.tile([128, 2 * NT], dt.float16, name="nsg_" + hname,
                              tag="nsg_" + hname)
                nc.scalar.activation(n_s[:], u[:], AF.Tanh)
                # dneg = nh_prev + n = n - h_prev ; e = z*dneg = -z*(h-n)
                # h' = n - e ... wait: h' = n + z*(h-n) = n - z*(n-h) = n - e
                dneg = gp.tile([128, 2 * NT], dt.float16, name="d_" + hname,
                               tag="d_g")
                e = gp.tile([128, 2 * NT], dt.float16, name="e_" + hname,
                            tag="e_g")
                h = sp.tile([128, 2 * NT], dt.float16, name=hname, tag=hname)
                nc.gpsimd.tensor_add(dneg[:], nh_prev[:], n_s[:])
                nc.gpsimd.tensor_mul(e[:], z_s[:], dneg[:])
                nc.vector.tensor_sub(h[:], n_s[:], e[:])
                return h

            def coord(i):
                ctile = xin.tile([4, NT], dt.float16, name="ctile", tag="ct")
                nc.sync.dma_start(ctile[:], ct_d[:, i * NT:(i + 1) * NT])
                cps = ps1("cps")  # [32, 2*NT] used: [z|n]
                nc.tensor.matmul(cps[0:32, 0:NT], wc[:, 32:64], ctile[:],
                                 start=True, stop=True)
                nc.tensor.matmul(cps[0:32, NT:2 * NT], wc[:, 64:96], ctile[:],
                                 start=True, stop=True)
                z_s = gp.tile([32, NT], dt.float16, name="czs", tag="czs")
                n_s = gp.tile([32, NT], dt.float16, name="cns", tag="cns")
                if zero_bias:
                    nc.scalar.activation(z_s[:], cps[0:32, 0:NT], AF.Sigmoid)
                    nc.scalar.activation(n_s[:], cps[0:32, NT:2 * NT], AF.Tanh)
                else:
                    nc.scalar.activation(z_s[:], cps[0:32, 0:NT], AF.Sigmoid,
                                         bias=biasc[:, 1:2])
                    rps = ps2("rps")
                    nc.tensor.matmul(rps[0:32, 0:NT], wc[:, 0:32], ctile[:],
                                     start=True, stop=True)
                    r_s = gp.tile([32, NT], dt.float16, name="crs", tag="crs")
                    nc.scalar.activation(r_s[:], rps[0:32, 0:NT], AF.Sigmoid,
                                         bias=biasc[:, 0:1])
                    tcd = gp.tile([32, NT], dt.float16, name="ctd", tag="ctd")
                    nc.vector.tensor_scalar_mul(tcd[:], r_s[:], biasc[:, 3:4])
                    ucd = gp.tile([32, NT], dt.float16, name="cud", tag="cud")
                    nc.vector.scalar_tensor_tensor(
                        ucd[:], cps[0:32, NT:2 * NT], biasc[:, 2:3], tcd[:],
                        OP.add, OP.add)
                    nc.scalar.activation(n_s[:], ucd[:], AF.Tanh)
                nhc = sp.tile([32, NT], dt.float16, name="nhc", tag="nhc")
                nc.vector.scalar_tensor_tensor(nhc[:], z_s[:], 1.0, n_s[:],
                                               OP.subtract, OP.mult)
                return nhc

            def fusion(i, h1, nhc):
                hid_ps = ps2("hid_ps")  # use [128, 0:NT]
                nc.tensor.matmul(hid_ps[:, 0:NT], wae[:, 0:128], h1[:, 0:NT],
                                 start=True, stop=False)
                nc.tensor.matmul(hid_ps[:, 0:NT], wae[:, 128:256],
                                 h1[:, NT:2 * NT], start=False, stop=False)
                nc.tensor.matmul(hid_ps[:, 0:NT], wacn[:], nhc[:],
                                 start=False, stop=True)
                hid = gp.tile([128, NT], dt.float16, name="hid", tag="hid")
                if zero_bias:
                    nc.vector.tensor_scalar(hid[:], hid_ps[:, 0:NT], 0.0, None,
                                            OP.max)
                else:
                    nc.vector.tensor_scalar(hid[:], hid_ps[:, 0:NT],
                                            bias[:, 16:17], 0.0,
                                            OP.add, OP.max)
                ops = ps1("ops")  # use [128, 0:4]
                for c in range(2):
                    nc.tensor.matmul(ops[:, c * 2:(c + 1) * 2],
                                     hid[:, c * 128:(c + 1) * 128], w2t[:],
                                     start=True, stop=True)
                outt = op_.tile([128, 4], dt.float32, name="outt", tag="outt")
                nc.vector.tensor_copy(outt[:], ops[:, 0:4])
                dst = out_d[:].rearrange("(a p) k -> p a k", p=128)
                nc.sync.dma_start(dst[:, 2 * i:2 * i + 2, :],
                                  outt[:].rearrange("p (c k) -> p c k", k=2))

            # ---- main loop -------------------------------------------
            for i in range(TILES):
                xt = xin.tile([128, NT], dt.float16, name="xt_t", tag="xt")
                nc.sync.dma_start(xt[:], xt_d[:, i * NT:(i + 1) * NT])
                nhc = coord(i)
                x0 = lambda kc: xt[0:64, :]
                x1 = lambda kc: xt[64:128, :]
                w0f0 = lambda kc, j: w0x[0:64, j * 128:(j + 1) * 128]
                w0f1 = lambda kc, j: w0x[64:128, j * 128:(j + 1) * 128]
                w1 = lambda kc, j: w1x[:, kc * G + j * 128:kc * G + (j + 1) * 128]
                w1n = lambda kc, j: w1xn[:, kc * G + j * 128:kc * G + (j + 1) * 128]
                nh0 = first_step(w0f0, x0, 1, 0, 8, 10, "nh01")
                nh1 = first_step(
                    w1n, lambda kc: nh0[:, kc * NT:(kc + 1) * NT], 2,
                    4, 12, 14, "nh11")
                h0_2 = full_step(w0f1, 1, x1, w0hn, nh0, 0, 8, 10, "h02")
                h1_2 = full_step(
                    w1, 2, lambda kc: h0_2[:, kc * NT:(kc + 1) * NT],
                    w1hn, nh1, 4, 12, 14, "h12")
                fusion(i, h1_2, nhc)

    nc.compile()
    return nc


def _prep_host(inputs):
    f32 = np.float32
    bf = np.asarray(inputs["batch_features"], dtype=f32)
    coords = np.asarray(inputs["coords"], dtype=f32)
    w = {k: np.asarray(inputs[k], dtype=f32) for k in inputs
         if k not in ("batch_features", "coords", "valid_mask")}

    XT = bf.transpose(1, 3, 0, 2).reshape(128, N_TOT)
    CT = coords.transpose(2, 0, 1).reshape(4, N_TOT)

    W1a, W1b = w["W1"][:, :128], w["W1"][:, 128:]
    wae = _pack_k(np.ascontiguousarray((W1a @ w["We"]).T), 128)
    wacn = np.ascontiguousarray(-(W1b @ w["Wc"]).T)
    b_hid = W1a @ w["be"] + W1b @ w["bc"] + w["b1"]

    def rzcols(b):  # (bih+bhh)[0:512] -> [128,4] cols r0 r1 z0 z1
        return np.ascontiguousarray(b[0:2 * HE].reshape(4, 128).T)

    bias = np.zeros((128, 18), f32)
    bias[:, 0:4] = rzcols(w["bih0"] + w["bhh0"])
    bias[:, 4:8] = rzcols(w["bih1"] + w["bhh1"])
    bias[:, 8:10] = w["bih0"][2 * HE:].reshape(2, 128).T
    bias[:, 10:12] = w["bhh0"][2 * HE:].reshape(2, 128).T
    bias[:, 12:14] = w["bih1"][2 * HE:].reshape(2, 128).T
    bias[:, 14:16] = w["bhh1"][2 * HE:].reshape(2, 128).T
    bias[:, 16] = b_hid
    biasc = np.zeros((32, 8), f32)
    bc_sum = w["bihC"] + w["bhhC"]
    biasc[:, 0] = bc_sum[0:HC]
    biasc[:, 1] = bc_sum[HC:2 * HC]
    biasc[:, 2] = w["bihC"][2 * HC:]
    biasc[:, 3] = w["bhhC"][2 * HC:]

    zero_bias = all(
        not np.any(w[k]) for k in
        ("bih0", "bhh0", "bih1", "bhh1", "bihC", "bhhC", "be", "bc", "b1"))

    w0xT = np.ascontiguousarray(w["Wih0"].T)
    w1xT = _pack_k(np.ascontiguousarray(w["Wih1"].T), G)
    wd = {
        "w0x": np.concatenate([w0xT, w0xT], axis=0).astype(F16),
        "w0hn": (-_pack_k(np.ascontiguousarray(w["Whh0"].T), G)).astype(F16),
        "w1x": w1xT.astype(F16),
        "w1xn": (-w1xT).astype(F16),
        "w1hn": (-_pack_k(np.ascontiguousarray(w["Whh1"].T), G)).astype(F16),
        "wc": np.ascontiguousarray(w["WihC"].T).astype(F16),
        "wae": wae.astype(F16),
        "wacn": wacn.astype(F16),
        "w2t": np.ascontiguousarray(w["W2"].T).astype(F16),
        "biases": bias,
        "biasc": biasc,
    }
    return XT, CT, wd, zero_bias, w["b2"]


def kernel(**inputs):
    global LAST_RESULTS
    from concourse.bass_utils import run_bass_kernel_spmd

    XT, CT, wd, zero_bias, b2 = _prep_host(inputs)

    key = ("v2", zero_bias)
    if key not in _CACHE:
        _CACHE[key] = _build_program(zero_bias)
    nc = _CACHE[key]

    in_maps = []
    for c in range(N_CORES):
        sl = slice(c * NC_TRACKS, (c + 1) * NC_TRACKS)
        m = dict(wd)
        m["xt"] = np.ascontiguousarray(XT[:, sl]).astype(F16)
        m["ct"] = np.ascontiguousarray(CT[:, sl]).astype(F16)
        in_maps.append(m)

    res = run_bass_kernel_spmd(nc, in_maps, list(range(N_CORES)))
    LAST_RESULTS = res

    out = np.concatenate([res.results[c]["out"] for c in range(N_CORES)], axis=0)
    out = out + b2[None, :].astype(np.float32)
    out = out.reshape(B, T, 2)
    mask = np.asarray(inputs["valid_mask"])
    return np.where(mask[:, :, None], out, np.float32(0.0)).astype(np.float32)
